# revision 1
# baseline (speedup 1.0000x reference)
"""Trainium2 Bass kernel for nn_ActELoss (windowed actioness similarity loss).

Reference computation (B=4096, T=750, window W=11, SIGMA=1):
    pad rows with 6 copies of first / 5 copies of last element, then
    loss = sum_{b,i,j<11} exp(-|a0[b,i] - a0[b,c(i+j-6)]|/2) * |a2[b,i] - a2[b,c(i+j-6)]|
         + 0.1 * sum_b ||a0[b] - a2[b]||_2
    with c(x) = clamp(x, 0, T-1).

Let f(i,j) = exp(-|a0_i-a0_j|/2)*|a2_i-a2_j| (symmetric, f(i,i)=0) and shift
s = j-6 in [-6, 4].  The s=0 term vanishes and term(i, s) = term(i+s, -s), so
the 11 shifts collapse to 6 interior diagonal sums
    I(k) = sum_{i=0}^{T-1-k} f(i, i+k),   k = 1..6
with weights 2,2,2,2,1,1, plus clamped-edge corrections:
    left:  sum_{i=1}^{5} (6-i) * f(i, 0)
    right: sum_{m=1}^{3} (4-m) * f(T-1-m, T-1)

Sharding: pure data parallel, 512 batch rows per core on 8 cores; each core
emits its partial main-loss scalar plus per-row ||a0-a2||^2 sums; the host
finishes sqrt over 4096 rows and the 8-way scalar all-reduce.

Implementation notes:
- Raw Bass blocks with hand-placed semaphores (not Tile): the walrus build in
  this container rejects instructions carrying more than one sync wait, so
  the schedule is constructed such that every instruction needs at most one.
- FLAT 2D layout [128 partitions, 4*750]: 4 batch rows concatenated per
  partition; shifts are free-dim offset slices.  Columns pairing across a
  row boundary compute garbage and are excluded from the reduction.
- Per shift k: DVE subtract + fused |.| (tensor_scalar abs_max), ACT exp
  (scale=-0.5), DVE subtract+abs for a2, DVE multiply into a bf16 product
  tile; the product is summed over (partitions x valid columns) by 1-column
  PE matmuls (lhsT = 2.0 for double-counted shifts k<=4 else 1.0)
  accumulating into one PSUM row of 512, collapsed at the end by a DVE
  tensor_reduce.
"""

import numpy as np

import concourse.bass as bass
from concourse import mybir
from concourse.bass_utils import run_bass_kernel_spmd

P = 128          # SBUF partitions
T = 750
B = 4096
N_CORES = 8
ROWS = B // N_CORES          # 512 rows per core
RP = ROWS // P               # 4 rows per partition
FW = RP * T                  # flat free width per partition
E_THETA = 0.1
PSUM_W = 512                 # accumulation row width (one PSUM bank)
NK = 6                       # interior shifts
_EDGE_W = RP * 5 + RP * 3    # per-row 5 left cols then 3 right cols

# compute dtype for the heavy elementwise work ("f32" or "bf16")
COMPUTE_DT = "bf16"

_F32 = mybir.dt.float32
_BF16 = mybir.dt.bfloat16


def build_nc(dt):
    nc = bass.Bass()
    op = mybir.AluOpType
    a0p = nc.declare_dram_parameter("a0", [P, FW], dt, isOutput=False)
    a2p = nc.declare_dram_parameter("a2", [P, FW], dt, isOutput=False)
    lossp = nc.declare_dram_parameter("loss", [1, 1], _F32, isOutput=True)
    normsqp = nc.declare_dram_parameter("normsq", [P, RP], _F32, isOutput=True)

    LW = RP * 5

    from contextlib import ExitStack

    with ExitStack() as ctx:
        a0f = ctx.enter_context(nc.sbuf_tensor([P, FW], dt))
        a2f = ctx.enter_context(nc.sbuf_tensor([P, FW], dt))
        d0A = ctx.enter_context(nc.sbuf_tensor([P, FW], dt))
        d0B = ctx.enter_context(nc.sbuf_tensor([P, FW], dt))
        d2A = ctx.enter_context(nc.sbuf_tensor([P, FW], dt))
        d2B = ctx.enter_context(nc.sbuf_tensor([P, FW], dt))
        wA = ctx.enter_context(nc.sbuf_tensor([P, FW], dt))
        wB = ctx.enter_context(nc.sbuf_tensor([P, FW], dt))
        dn = ctx.enter_context(nc.sbuf_tensor([P, FW], dt))
        dn2 = ctx.enter_context(nc.sbuf_tensor([P, FW], dt))
        prods = ctx.enter_context(nc.sbuf_tensor([P, NK + 1, FW], _BF16))
        e0 = ctx.enter_context(nc.sbuf_tensor([P, _EDGE_W], dt))
        e2 = ctx.enter_context(nc.sbuf_tensor([P, _EDGE_W], dt))
        we = ctx.enter_context(nc.sbuf_tensor([P, _EDGE_W], dt))
        coeffE = ctx.enter_context(nc.sbuf_tensor([P, _EDGE_W], dt))
        ones = ctx.enter_context(nc.sbuf_tensor([P, 1], _BF16))
        twos = ctx.enter_context(nc.sbuf_tensor([P, 1], _BF16))
        accN = ctx.enter_context(nc.sbuf_tensor([P, RP], _F32))
        res = ctx.enter_context(nc.sbuf_tensor([1, 1], _F32))
        d2aA = ctx.enter_context(nc.sbuf_tensor([P, FW], dt))
        d2aB = ctx.enter_context(nc.sbuf_tensor([P, FW], dt))
        negA = ctx.enter_context(nc.sbuf_tensor([P, FW], dt))
        negB = ctx.enter_context(nc.sbuf_tensor([P, FW], dt))
        warm = ctx.enter_context(nc.sbuf_tensor([1, 1], dt))
        warmdst = ctx.enter_context(nc.sbuf_tensor([1, 1], dt))
        ps = ctx.enter_context(nc.psum_tensor([1, PSUM_W], _F32))
        dma_sem = ctx.enter_context(nc.semaphore("dma_sem"))
        dve_sem = ctx.enter_context(nc.semaphore("dve_sem"))
        dve2_sem = ctx.enter_context(nc.semaphore("dve2_sem"))
        act_sem = ctx.enter_context(nc.semaphore("act_sem"))
        acts_sem = ctx.enter_context(nc.semaphore("acts_sem"))
        dves_sem = ctx.enter_context(nc.semaphore("dves_sem"))
        pe_sem = ctx.enter_context(nc.semaphore("pe_sem"))
        gps_sem = ctx.enter_context(nc.semaphore("gps_sem"))
        block = ctx.enter_context(nc.Block())
        d0 = [d0A, d0B]
        d2 = [d2A, d2B]
        wt = [wA, wB]
        d2a = [d2aA, d2aB]
        negT = [negA, negB]
        MOVED = (5, 6)   # shifts whose |d2| runs on DVE instead of ACT
        Abs = mybir.ActivationFunctionType.Abs
        Exp = mybir.ActivationFunctionType.Exp

        # dve_sem milestones (inc'd in DVE program order): 2k-1 = sub0_k,
        # 2k = sub2_k (k=1..6), 13 = accN ready, 14 = edge e0 diffs done,
        # 15 = edge e2 diffs done, 16 = res ready.
        # dve2_sem: k = prod_k ready (k=1..6), 7 = edge product ready.
        # act_sem: k = shift-k ACT chain (abs0, exp, abs2) done, 7 = edge
        # ACT work done.  pe_sem: 1 = all accumulation matmuls retired.

        @block.sync
        def _(sync):
            sync.dma_start(out=a0f[:, :], in_=a0p[:, :]).then_inc(dma_sem, 16)
            sync.dma_start(out=a2f[:, :], in_=a2p[:, :]).then_inc(dma_sem, 16)
            sync.wait_ge(dve_sem, 13)
            sync.dma_start(out=normsqp[:, :], in_=accN[:, :]).then_inc(dma_sem, 16)
            sync.wait_ge(dve_sem, 16)
            sync.dma_start(out=lossp[:, :], in_=res[:, :]).then_inc(dma_sem, 16)

        @block.vector
        def _(vector):
            # warmup source for the early ACT table-set load (no data deps)
            vector.memset(warm[:, :], 0.0).then_inc(dves_sem, 1)
            # constants (no data deps)
            vector.memset(ones[:, :], 1.0)
            vector.memset(twos[:, :], 2.0)
            ce = coeffE[:, :]
            for j, v in enumerate((5.0, 4.0, 3.0, 2.0, 1.0)):
                vector.memset(
                    bass.AP(tensor=ce.tensor, offset=coeffE[:, j : j + 1].offset,
                            ap=[ce.ap[0], [5, RP]]), v)
            for j, v in enumerate((1.0, 2.0, 3.0)):
                vector.memset(
                    bass.AP(tensor=ce.tensor,
                            offset=coeffE[:, LW + j : LW + j + 1].offset,
                            ap=[ce.ap[0], [3, RP]]), v)

            vector.wait_ge(dma_sem, 32)

            def subs(k):
                FL = FW - k
                vector.tensor_tensor(
                    out=d0[k % 2][:, :FL], in0=a0f[:, :FL], in1=a0f[:, k:],
                    op=op.subtract,
                ).then_inc(dve_sem, 1)          # 2k-1: d0_k ready (feeds ACT)
                vector.tensor_tensor(
                    out=d2[k % 2][:, :FL], in0=a2f[:, :FL], in1=a2f[:, k:],
                    op=op.subtract,
                ).then_inc(dve_sem, 1)          # 2k: d2_k ready

            def mul(k):
                FL = FW - k
                if k in MOVED:
                    # |d2| on DVE: negate (4x tensor_scalar) then max.  The
                    # act>=k wait rides on the neg (it covers sub2_k because
                    # the moved shifts' abs0 waits dve>=2k); the neg->max and
                    # max->mul chains thread through dves_sem.
                    i = MOVED.index(k)
                    vector.wait_ge(act_sem, k)
                    vector.tensor_scalar(
                        out=negT[k % 2][:, :FL], in0=d2[k % 2][:, :FL],
                        scalar1=-1.0, scalar2=None, op0=op.mult,
                    ).then_inc(dves_sem, 1)     # 2 + 2i
                    vector.wait_ge(dves_sem, 2 + 2 * i)
                    vector.tensor_tensor(
                        out=d2a[k % 2][:, :FL], in0=d2[k % 2][:, :FL],
                        in1=negT[k % 2][:, :FL], op=op.max,
                    ).then_inc(dves_sem, 1)     # 3 + 2i
                    vector.wait_ge(dves_sem, 3 + 2 * i)
                else:
                    vector.wait_ge(act_sem, k)
                vector.tensor_tensor(
                    out=prods[:, k - 1, :FL], in0=wt[k % 2][:, :FL],
                    in1=d2a[k % 2][:, :FL], op=op.mult,
                ).then_inc(dve2_sem, 1)         # k: prod_k ready

            # software-pipelined by one shift: shift k+1's subtracts issue
            # before shift k's multiply, so DVE keeps busy while ACT runs
            # the abs/exp chain for shift k
            subs(1)
            for k in range(1, NK):
                subs(k + 1)
                mul(k)
                if k == NK - 1:
                    # norm row-reductions: dn2 was produced by GPSIMD while
                    # the shifts ran; this slot fills DVE's wait on the last
                    # ACT chain without delaying anything ACT needs
                    vector.wait_ge(gps_sem, 2)
                    for t in range(RP):
                        inst = vector.tensor_reduce(
                            out=accN[:, t : t + 1],
                            in_=dn2[:, t * T : (t + 1) * T],
                            op=op.add, axis=mybir.AxisListType.X,
                        )
                    inst.then_inc(dve_sem, 1)   # 13: accN ready
            mul(NK)

            # edge diffs (x - edge_col) via broadcast (stride-0) subtract
            def bfree(ap1, n):
                return bass.AP(tensor=ap1.tensor, offset=ap1.offset,
                               ap=[*ap1.ap[:-1], [0, n]])

            for t in range(RP):
                vector.tensor_tensor(
                    out=e0[:, t * 5 : (t + 1) * 5],
                    in0=a0f[:, t * T + 1 : t * T + 6],
                    in1=bfree(a0f[:, t * T : t * T + 1], 5), op=op.subtract,
                )
                inst = vector.tensor_tensor(
                    out=e0[:, LW + t * 3 : LW + (t + 1) * 3],
                    in0=a0f[:, t * T + 746 : t * T + 749],
                    in1=bfree(a0f[:, t * T + 749 : t * T + 750], 3), op=op.subtract,
                )
            inst.then_inc(dve_sem, 1)           # 14: e0 diffs ready
            for t in range(RP):
                vector.tensor_tensor(
                    out=e2[:, t * 5 : (t + 1) * 5],
                    in0=a2f[:, t * T + 1 : t * T + 6],
                    in1=bfree(a2f[:, t * T : t * T + 1], 5), op=op.subtract,
                )
                inst = vector.tensor_tensor(
                    out=e2[:, LW + t * 3 : LW + (t + 1) * 3],
                    in0=a2f[:, t * T + 746 : t * T + 749],
                    in1=bfree(a2f[:, t * T + 749 : t * T + 750], 3), op=op.subtract,
                )
            inst.then_inc(dve_sem, 1)           # 15: e2 diffs ready
            vector.wait_ge(act_sem, NK + 1)
            vector.tensor_tensor(
                out=e2[:, :], in0=e2[:, :], in1=coeffE[:, :], op=op.mult
            ).then_inc(dves_sem, 1)
            vector.wait_ge(dves_sem, 6)
            vector.tensor_tensor(
                out=prods[:, NK, :_EDGE_W], in0=we[:, :], in1=e2[:, :], op=op.mult
            ).then_inc(dve2_sem, 1)             # 7: edge product ready

            vector.wait_ge(pe_sem, 1)
            vector.tensor_reduce(
                out=res[:, :], in_=ps[:, :], op=op.add, axis=mybir.AxisListType.X
            ).then_inc(dve_sem, 1)              # 16: res ready

        @block.scalar
        def _(scalar):
            # warmup: trigger the exp/abs table-set load while the input
            # DMAs are still in flight
            scalar.wait_ge(dves_sem, 1)
            scalar.activation(out=warmdst[:, :], in_=warm[:, :], func=Exp)
            # ACT has no inter-op drain: dependent back-to-back ACT ops need
            # an explicit self-semaphore (acts_sem) between write and read.
            for k in range(1, NK + 1):
                FL = FW - k
                scalar.wait_ge(dve_sem, 2 * k if k in (5, 6) else 2 * k - 1)
                scalar.activation(out=d0[k % 2][:, :FL], in_=d0[k % 2][:, :FL],
                                  func=Abs).then_inc(acts_sem, 1)
                scalar.wait_ge(acts_sem, k)
                if k in (5, 6):
                    scalar.activation(out=wt[k % 2][:, :FL],
                                      in_=d0[k % 2][:, :FL],
                                      func=Exp, scale=-0.5).then_inc(act_sem, 1)
                else:
                    scalar.activation(out=wt[k % 2][:, :FL],
                                      in_=d0[k % 2][:, :FL],
                                      func=Exp, scale=-0.5)
                    scalar.wait_ge(dve_sem, 2 * k)
                    scalar.activation(out=d2a[k % 2][:, :FL],
                                      in_=d2[k % 2][:, :FL],
                                      func=Abs).then_inc(act_sem, 1)
            scalar.wait_ge(dve_sem, 14)
            scalar.activation(out=e0[:, :], in_=e0[:, :],
                              func=Abs).then_inc(acts_sem, 1)
            scalar.wait_ge(acts_sem, NK + 1)
            scalar.activation(out=we[:, :], in_=e0[:, :], func=Exp, scale=-0.5)
            scalar.wait_ge(dve_sem, 15)
            scalar.activation(out=e2[:, :], in_=e2[:, :],
                              func=Abs).then_inc(act_sem, 1)

        @block.gpsimd
        def _(gp):
            gp.wait_ge(dma_sem, 32)
            gp.tensor_tensor(
                out=dn[:, :], in0=a0f[:, :], in1=a2f[:, :], op=op.subtract
            ).then_inc(gps_sem, 1)
            gp.wait_ge(gps_sem, 1)
            gp.tensor_tensor(
                out=dn2[:, :], in0=dn[:, :], in1=dn[:, :], op=op.mult
            ).then_inc(gps_sem, 1)

        @block.tensor
        def _(tensor):
            started = False
            for k in range(1, NK + 1):
                tensor.wait_ge(dve2_sem, k)
                lhsT = twos if k <= 4 else ones
                for t in range(RP):
                    base = t * T
                    width = T - k
                    for coff in range(0, width, PSUM_W):
                        cw = min(PSUM_W, width - coff)
                        tensor.matmul(
                            ps[:, :cw], lhsT[:, :],
                            prods[:, k - 1, base + coff : base + coff + cw],
                            start=not started, stop=False,
                        )
                        started = True
            tensor.wait_ge(dve2_sem, NK + 1)
            tensor.matmul(
                ps[:, :_EDGE_W], ones[:, :], prods[:, NK, :_EDGE_W],
                start=False, stop=True,
            ).then_inc(pe_sem, 1)

    return nc


_CACHE = {}


def _get_nc():
    if COMPUTE_DT not in _CACHE:
        dt = _F32 if COMPUTE_DT == "f32" else _BF16
        _CACHE[COMPUTE_DT] = (build_nc(dt), dt)
    return _CACHE[COMPUTE_DT]


def _run(actioness, actioness_2, **spmd_kwargs):
    nc, dt = _get_nc()
    np_dt = mybir.dt.np(dt)
    a0 = np.ascontiguousarray(actioness, dtype=np.float32)
    a2 = np.ascontiguousarray(actioness_2, dtype=np.float32)
    def perm(arr):
        return np.ascontiguousarray(
            arr.reshape(RP, P, T).transpose(1, 0, 2).reshape(P, FW)
        ).astype(np_dt)

    in_maps = []
    for c in range(N_CORES):
        sl = slice(c * ROWS, (c + 1) * ROWS)
        in_maps.append({"a0": perm(a0[sl]), "a2": perm(a2[sl])})
    res = run_bass_kernel_spmd(nc, in_maps, list(range(N_CORES)), **spmd_kwargs)
    total = 0.0
    for r in res.results:
        total += float(r["loss"][0, 0])
        total += E_THETA * float(np.sqrt(r["normsq"].astype(np.float64)).sum())
    return np.float32(total), res


def kernel(actioness, actioness_2):
    out, _ = _run(actioness, actioness_2)
    return out



# revision 2
# speedup vs baseline: 1.0064x; 1.0064x over previous
"""Trainium2 Bass kernel v4 for nn_ActELoss.

Same math as v2/v3.  Changes vs v3:
- pr triple-buffered (PE WAR guard k-3 -> no DVE stall on PE).
- shift-1 split asymmetric (small first chunk [0:1002]) so the ACT abs/exp
  chain starts ~2us earlier; a0 DMA split to match.
- shift-6 split asymmetric (small second chunk [2016:3018]) to shorten the
  dependency tail after the last exp.
- final PSUM collapse back on DVE (tensor_reduce is faster than ACT copy).
"""

import numpy as np

import concourse.bass as bass
from concourse import mybir
from concourse.bass_utils import run_bass_kernel_spmd

P = 128
T = 750
B = 4096
N_CORES = 8
ROWS = B // N_CORES
RP = ROWS // P
TP = T + 6
FWH = RP * TP                # 3024
W3 = FWH - 6                 # 3018
S1 = 1002                    # shift-1 first-chunk op width
S1C = 1008                   # first-half DMA column count
S6 = 2016                    # shift-6 first-chunk op width
NK = 6
CK = (2.0, 2.0, 2.0, 2.0, 1.0, 1.0)
E_THETA = 0.1
PAD0 = 100.0
PSUM_W = 512

_F32 = mybir.dt.float32
_BF16 = mybir.dt.bfloat16


def build_nc():
    nc = bass.Bass()
    op = mybir.AluOpType
    Abs = mybir.ActivationFunctionType.Abs
    Exp = mybir.ActivationFunctionType.Exp

    a0p = nc.declare_dram_parameter("a0", [P, FWH], _BF16, isOutput=False)
    a2p = nc.declare_dram_parameter("a2", [P, FWH], _BF16, isOutput=False)
    resp = nc.declare_dram_parameter("res", [1, 1], _F32, isOutput=True)

    from contextlib import ExitStack

    with ExitStack() as ctx:
        a0f = ctx.enter_context(nc.sbuf_tensor([P, FWH], _BF16))
        a2f = ctx.enter_context(nc.sbuf_tensor([P, FWH], _BF16))
        dzs = [ctx.enter_context(nc.sbuf_tensor(f"dz{i}", [P, W3], _BF16))
               for i in range(NK)]
        Ws = [ctx.enter_context(nc.sbuf_tensor(f"W{i}", [P, W3], _BF16))
              for i in range(NK)]
        dsA = ctx.enter_context(nc.sbuf_tensor([P, W3], _BF16))
        dsB = ctx.enter_context(nc.sbuf_tensor([P, W3], _BF16))
        prs = [ctx.enter_context(nc.sbuf_tensor(f"pr{i}", [P, 2 * W3], _BF16))
               for i in range(3)]
        ones = ctx.enter_context(nc.sbuf_tensor([P, 1], _BF16))
        biasln2 = ctx.enter_context(nc.sbuf_tensor([P, 1], _F32))
        res = ctx.enter_context(nc.sbuf_tensor([1, 1], _F32))
        warm = ctx.enter_context(nc.sbuf_tensor([1, 1], _BF16))
        warmdst = ctx.enter_context(nc.sbuf_tensor([1, 1], _BF16))
        ps = ctx.enter_context(nc.psum_tensor([1, PSUM_W], _F32))
        dma_sem = ctx.enter_context(nc.semaphore("dma_sem"))
        ws_sem = ctx.enter_context(nc.semaphore("ws_sem"))
        v_sem = ctx.enter_context(nc.semaphore("v_sem"))
        a_sem = ctx.enter_context(nc.semaphore("a_sem"))
        aa_sem = ctx.enter_context(nc.semaphore("aa_sem"))
        pe_sem = ctx.enter_context(nc.semaphore("pe_sem"))
        block = ctx.enter_context(nc.Block())
        ds = [dsB, dsA]

        # v_sem: sub0_1a=1 sub0_1b=2 sub0_2..6=3..7 relu_1..5=8..12
        #        relu_6a=13 relu_6b=14 res=15
        # a_sem: exp_1a=1 exp_1b=2 exp_2..5=3..6 exp_6a=7 exp_6b=8
        # aa_sem: abs ops 1..8.  pe_sem: sets 1..5 -> 1..5, 6a=6, 6b=7

        @block.sync
        def _(sync):
            sync.dma_start(out=a0f[:, :S1C], in_=a0p[:, :S1C]).then_inc(dma_sem, 16)
            sync.dma_start(out=a0f[:, S1C:], in_=a0p[:, S1C:]).then_inc(dma_sem, 16)
            sync.dma_start(out=a2f[:, :], in_=a2p[:, :]).then_inc(dma_sem, 16)
            sync.wait_ge(v_sem, 15)
            sync.dma_start(out=resp[:, :], in_=res[:, :]).then_inc(dma_sem, 16)

        @block.vector
        def _(vector):
            vector.memset(ones[:, :], 1.0)
            vector.memset(biasln2[:, :], float(np.log(2.0)))
            vector.memset(warm[:, :], 0.0).then_inc(ws_sem, 1)
            vector.wait_ge(dma_sem, 16)
            vector.tensor_tensor(
                out=dzs[0][:, :S1], in0=a0f[:, :S1],
                in1=a0f[:, 1:1 + S1], op=op.subtract,
            ).then_inc(v_sem, 1)                        # v1
            vector.wait_ge(dma_sem, 32)
            vector.tensor_tensor(
                out=dzs[0][:, S1:], in0=a0f[:, S1:W3],
                in1=a0f[:, S1 + 1:W3 + 1], op=op.subtract,
            ).then_inc(v_sem, 1)                        # v2
            for k in range(2, NK + 1):
                vector.tensor_tensor(
                    out=dzs[k - 1][:, :], in0=a0f[:, :W3],
                    in1=a0f[:, k:k + W3], op=op.subtract,
                ).then_inc(v_sem, 1)                    # v3..v7
            for k in range(1, NK + 1):
                pb = prs[k % 3]
                if k == 1:
                    vector.wait_ge(dma_sem, 48)
                elif k >= 4:
                    vector.wait_ge(pe_sem, k - 3)       # pr WAR guard
                vector.tensor_tensor(
                    out=ds[k % 2][:, :], in0=a2f[:, k:k + W3],
                    in1=a2f[:, :W3], op=op.subtract,
                )
                if k < NK:
                    vector.wait_ge(a_sem, k + 1)        # W_k ready
                    vector.tensor_tensor(
                        out=pb[:, :W3], in0=Ws[k - 1][:, :],
                        in1=ds[k % 2][:, :], op=op.mult,
                    )
                    vector.tensor_scalar(
                        out=pb[:, W3:], in0=pb[:, :W3],
                        scalar1=-2.0, scalar2=0.0, op0=op.mult, op1=op.max,
                    ).then_inc(v_sem, 1)                # v8..v12
                else:
                    vector.wait_ge(a_sem, 7)
                    vector.tensor_tensor(
                        out=pb[:, :S6], in0=Ws[5][:, :S6],
                        in1=ds[0][:, :S6], op=op.mult,
                    )
                    vector.tensor_scalar(
                        out=pb[:, W3:W3 + S6], in0=pb[:, :S6],
                        scalar1=-2.0, scalar2=0.0, op0=op.mult, op1=op.max,
                    ).then_inc(v_sem, 1)                # v13
                    vector.wait_ge(a_sem, 8)
                    vector.tensor_tensor(
                        out=pb[:, S6:W3], in0=Ws[5][:, S6:],
                        in1=ds[0][:, S6:], op=op.mult,
                    )
                    vector.tensor_scalar(
                        out=pb[:, W3 + S6:], in0=pb[:, S6:W3],
                        scalar1=-2.0, scalar2=0.0, op0=op.mult, op1=op.max,
                    ).then_inc(v_sem, 1)                # v14
            vector.wait_ge(pe_sem, 7)
            vector.tensor_reduce(
                out=res[:, :], in_=ps[:, :], op=op.add,
                axis=mybir.AxisListType.X,
            ).then_inc(v_sem, 1)                        # v15

        @block.scalar
        def _(scalar):
            scalar.wait_ge(ws_sem, 1)
            scalar.activation(out=warmdst[:, :], in_=warm[:, :], func=Exp)

            def absexp(src_slice, w_slice, vwait, aa, bias):
                if vwait is not None:
                    scalar.wait_ge(v_sem, vwait)
                scalar.activation(
                    out=src_slice, in_=src_slice, func=Abs,
                ).then_inc(aa_sem, 1)
                scalar.wait_ge(aa_sem, aa)
                scalar.activation(
                    out=w_slice, in_=src_slice, func=Exp, scale=-0.5, bias=bias,
                ).then_inc(a_sem, 1)

            b2 = biasln2[:, :]
            absexp(dzs[0][:, :S1], Ws[0][:, :S1], 1, 1, b2)
            absexp(dzs[0][:, S1:], Ws[0][:, S1:], 2, 2, b2)
            for k in range(2, NK):
                absexp(dzs[k - 1][:, :], Ws[k - 1][:, :], k + 1,
                       k + 1, b2 if CK[k - 1] == 2.0 else 0.0)
            absexp(dzs[5][:, :S6], Ws[5][:, :S6], 7, 7, 0.0)
            absexp(dzs[5][:, S6:], Ws[5][:, S6:], None, 8, 0.0)

        @block.tensor
        def _(tensor):
            tensor.wait_ge(ws_sem, 1)
            started = False

            def mmset(buf, ranges, vwait, stop_set=False):
                nonlocal started
                tensor.wait_ge(v_sem, vwait)
                chunks = []
                for lo, hi in ranges:
                    for coff in range(lo, hi, PSUM_W):
                        chunks.append((coff, min(PSUM_W, hi - coff)))
                for i, (coff, cw) in enumerate(chunks):
                    inst = tensor.matmul(
                        ps[:, :cw], ones[:, :], buf[:, coff:coff + cw],
                        start=not started,
                        stop=(stop_set and i == len(chunks) - 1),
                    )
                    started = True
                inst.then_inc(pe_sem, 1)

            for k in range(1, NK):
                mmset(prs[k % 3], [(0, 2 * W3)], 7 + k)
            mmset(prs[0], [(0, S6), (W3, W3 + S6)], 13)
            mmset(prs[0], [(S6, W3), (W3 + S6, 2 * W3)], 14, stop_set=True)

    return nc


_CACHE = {}


def _get_nc():
    if "nc" not in _CACHE:
        _CACHE["nc"] = build_nc()
    return _CACHE["nc"]


def _perm_pad(arr, pad):
    out = np.full((ROWS, TP), pad, dtype=np.float32)
    out[:, :T] = arr
    out = out.reshape(RP, P, TP).transpose(1, 0, 2).reshape(P, FWH)
    return np.ascontiguousarray(out).astype(mybir.dt.np(_BF16))


def _run(actioness, actioness_2, **spmd_kwargs):
    nc = _get_nc()
    a0 = np.ascontiguousarray(actioness, dtype=np.float32)
    a2 = np.ascontiguousarray(actioness_2, dtype=np.float32)
    in_maps = []
    for c in range(N_CORES):
        sl = slice(c * ROWS, (c + 1) * ROWS)
        in_maps.append({
            "a0": _perm_pad(a0[sl], PAD0),
            "a2": _perm_pad(a2[sl], 0.0),
        })
    res = run_bass_kernel_spmd(nc, in_maps, list(range(N_CORES)), **spmd_kwargs)

    total = 0.0
    for r in res.results:
        total += float(r["res"][0, 0])

    # norm + clamped-edge corrections on host in fp64 (~4% of the FLOPs)
    a0d = a0.astype(np.float64)
    a2d = a2.astype(np.float64)
    total += E_THETA * float(np.sum(np.sqrt(np.sum((a0d - a2d) ** 2, axis=1))))

    def f(i, j):
        return np.exp(-np.abs(a0d[:, i] - a0d[:, j]) / 2.0) * np.abs(
            a2d[:, i] - a2d[:, j])

    for i in range(1, 6):
        total += (6 - i) * float(np.sum(f(i, 0)))
    for m in range(1, 4):
        total += (4 - m) * float(np.sum(f(T - 1 - m, T - 1)))
    return np.float32(total), res


def kernel(actioness, actioness_2):
    out, _ = _run(actioness, actioness_2)
    return out


# revision 3
# speedup vs baseline: 1.0258x; 1.0193x over previous
"""Trainium2 Bass kernel v4 for nn_ActELoss.

Same math as v2/v3.  Changes vs v3:
- pr triple-buffered (PE WAR guard k-3 -> no DVE stall on PE).
- shift-1 split asymmetric (small first chunk [0:1002]) so the ACT abs/exp
  chain starts ~2us earlier; a0 DMA split to match.
- shift-6 split asymmetric (small second chunk [2016:3018]) to shorten the
  dependency tail after the last exp.
- final PSUM collapse back on DVE (tensor_reduce is faster than ACT copy).
"""

import numpy as np

import concourse.bass as bass
from concourse import mybir
from concourse.bass_utils import run_bass_kernel_spmd

P = 128
T = 750
B = 4096
N_CORES = 8
ROWS = B // N_CORES
RP = ROWS // P
TP = T + 6
FWH = RP * TP                # 3024
W3 = FWH - 6                 # 3018
S1 = 1002                    # shift-1 first-chunk op width
S1C = 1008                   # first-half DMA column count
S6 = 2016                    # shift-6 first-chunk op width
NK = 6
CK = (2.0, 2.0, 2.0, 2.0, 1.0, 1.0)
E_THETA = 0.1
PAD0 = 100.0
PSUM_W = 512

_F32 = mybir.dt.float32
_BF16 = mybir.dt.bfloat16


def build_nc():
    nc = bass.Bass()
    op = mybir.AluOpType
    Abs = mybir.ActivationFunctionType.Abs
    Exp = mybir.ActivationFunctionType.Exp

    a0p = nc.declare_dram_parameter("a0", [P, FWH], _BF16, isOutput=False)
    a2p = nc.declare_dram_parameter("a2", [P, FWH], _BF16, isOutput=False)
    resp = nc.declare_dram_parameter("res", [1, 1], _F32, isOutput=True)

    from contextlib import ExitStack

    with ExitStack() as ctx:
        a0f = ctx.enter_context(nc.sbuf_tensor([P, FWH], _BF16))
        a2f = ctx.enter_context(nc.sbuf_tensor([P, FWH], _BF16))
        dzs = [ctx.enter_context(nc.sbuf_tensor(f"dz{i}", [P, W3], _BF16))
               for i in range(NK)]
        Ws = [ctx.enter_context(nc.sbuf_tensor(f"W{i}", [P, W3], _BF16))
              for i in range(NK)]
        dsA = ctx.enter_context(nc.sbuf_tensor([P, W3], _BF16))
        dsB = ctx.enter_context(nc.sbuf_tensor([P, W3], _BF16))
        prs = [ctx.enter_context(nc.sbuf_tensor(f"pr{i}", [P, 2 * W3], _BF16))
               for i in range(3)]
        ones = ctx.enter_context(nc.sbuf_tensor([P, 1], _BF16))
        biasln2 = ctx.enter_context(nc.sbuf_tensor([P, 1], _F32))
        res = ctx.enter_context(nc.sbuf_tensor([1, 1], _F32))
        warm = ctx.enter_context(nc.sbuf_tensor([1, 1], _BF16))
        warmdst = ctx.enter_context(nc.sbuf_tensor([1, 1], _BF16))
        ps = ctx.enter_context(nc.psum_tensor([1, PSUM_W], _F32))
        dma_sem = ctx.enter_context(nc.semaphore("dma_sem"))
        ws_sem = ctx.enter_context(nc.semaphore("ws_sem"))
        v_sem = ctx.enter_context(nc.semaphore("v_sem"))
        a_sem = ctx.enter_context(nc.semaphore("a_sem"))
        aa_sem = ctx.enter_context(nc.semaphore("aa_sem"))
        pe_sem = ctx.enter_context(nc.semaphore("pe_sem"))
        block = ctx.enter_context(nc.Block())
        ds = [dsB, dsA]

        # v_sem: sub0_1a=1 sub0_1b=2 sub0_2..6=3..7 relu_1..5=8..12
        #        relu_6a=13 relu_6b=14 res=15
        # a_sem: exp_1a=1 exp_1b=2 exp_2..5=3..6 exp_6a=7 exp_6b=8
        # aa_sem: abs ops 1..8.  pe_sem: sets 1..5 -> 1..5, 6a=6, 6b=7

        @block.sync
        def _(sync):
            sync.dma_start(out=a0f[:, :S1C], in_=a0p[:, :S1C]).then_inc(dma_sem, 16)
            sync.dma_start(out=a0f[:, S1C:], in_=a0p[:, S1C:]).then_inc(dma_sem, 16)
            sync.dma_start(out=a2f[:, :], in_=a2p[:, :]).then_inc(dma_sem, 16)
            sync.wait_ge(v_sem, 15)
            sync.dma_start(out=resp[:, :], in_=res[:, :]).then_inc(dma_sem, 16)

        @block.vector
        def _(vector):
            vector.memset(ones[:, :], 1.0)
            vector.memset(biasln2[:, :], float(np.log(2.0)))
            vector.memset(warm[:, :], 0.0).then_inc(ws_sem, 1)
            vector.wait_ge(dma_sem, 16)
            vector.tensor_tensor(
                out=dzs[0][:, :S1], in0=a0f[:, :S1],
                in1=a0f[:, 1:1 + S1], op=op.subtract,
            ).then_inc(v_sem, 1)                        # v1
            vector.wait_ge(dma_sem, 32)
            vector.tensor_tensor(
                out=dzs[0][:, S1:], in0=a0f[:, S1:W3],
                in1=a0f[:, S1 + 1:W3 + 1], op=op.subtract,
            ).then_inc(v_sem, 1)                        # v2
            for k in range(2, NK + 1):
                vector.tensor_tensor(
                    out=dzs[k - 1][:, :], in0=a0f[:, :W3],
                    in1=a0f[:, k:k + W3], op=op.subtract,
                ).then_inc(v_sem, 1)                    # v3..v7
            for k in range(1, NK + 1):
                pb = prs[k % 3]
                if k == 1:
                    vector.wait_ge(dma_sem, 48)
                elif k >= 4:
                    vector.wait_ge(pe_sem, k - 3)       # pr WAR guard
                vector.tensor_tensor(
                    out=ds[k % 2][:, :], in0=a2f[:, k:k + W3],
                    in1=a2f[:, :W3], op=op.subtract,
                )
                if k < NK:
                    vector.wait_ge(a_sem, k + 1)        # W_k ready
                    vector.tensor_tensor(
                        out=pb[:, :W3], in0=Ws[k - 1][:, :],
                        in1=ds[k % 2][:, :], op=op.mult,
                    )
                    vector.tensor_scalar(
                        out=pb[:, W3:], in0=pb[:, :W3],
                        scalar1=-2.0, scalar2=0.0, op0=op.mult, op1=op.max,
                    ).then_inc(v_sem, 1)                # v8..v12
                else:
                    vector.wait_ge(a_sem, 7)
                    vector.tensor_tensor(
                        out=pb[:, :S6], in0=Ws[5][:, :S6],
                        in1=ds[0][:, :S6], op=op.mult,
                    )
                    vector.tensor_scalar(
                        out=pb[:, W3:W3 + S6], in0=pb[:, :S6],
                        scalar1=-2.0, scalar2=0.0, op0=op.mult, op1=op.max,
                    ).then_inc(v_sem, 1)                # v13
                    vector.wait_ge(a_sem, 8)
                    vector.tensor_tensor(
                        out=pb[:, S6:W3], in0=Ws[5][:, S6:],
                        in1=ds[0][:, S6:], op=op.mult,
                    )
                    vector.tensor_scalar(
                        out=pb[:, W3 + S6:], in0=pb[:, S6:W3],
                        scalar1=-2.0, scalar2=0.0, op0=op.mult, op1=op.max,
                    ).then_inc(v_sem, 1)                # v14
            vector.wait_ge(pe_sem, 7)
            vector.tensor_reduce(
                out=res[:, :], in_=ps[:, :], op=op.add,
                axis=mybir.AxisListType.X,
            ).then_inc(v_sem, 1)                        # v15

        @block.scalar
        def _(scalar):
            scalar.wait_ge(ws_sem, 1)
            scalar.activation(out=warmdst[:, :], in_=warm[:, :], func=Exp)

            def absexp(src_slice, w_slice, vwait, aa, bias):
                if vwait is not None:
                    scalar.wait_ge(v_sem, vwait)
                scalar.activation(
                    out=src_slice, in_=src_slice, func=Abs,
                ).then_inc(aa_sem, 1)
                scalar.wait_ge(aa_sem, aa)
                scalar.activation(
                    out=w_slice, in_=src_slice, func=Exp, scale=-0.5, bias=bias,
                ).then_inc(a_sem, 1)

            b2 = biasln2[:, :]
            absexp(dzs[0][:, :S1], Ws[0][:, :S1], 1, 1, b2)
            absexp(dzs[0][:, S1:], Ws[0][:, S1:], 2, 2, b2)
            for k in range(2, NK):
                absexp(dzs[k - 1][:, :], Ws[k - 1][:, :], k + 1,
                       k + 1, b2 if CK[k - 1] == 2.0 else 0.0)
            absexp(dzs[5][:, :S6], Ws[5][:, :S6], 7, 7, 0.0)
            absexp(dzs[5][:, S6:], Ws[5][:, S6:], None, 8, 0.0)

        @block.tensor
        def _(tensor):
            tensor.wait_ge(ws_sem, 1)
            started = False

            def mmset(buf, ranges, vwait, stop_set=False):
                nonlocal started
                tensor.wait_ge(v_sem, vwait)
                chunks = []
                for lo, hi in ranges:
                    for coff in range(lo, hi, PSUM_W):
                        chunks.append((coff, min(PSUM_W, hi - coff)))
                for i, (coff, cw) in enumerate(chunks):
                    inst = tensor.matmul(
                        ps[:, :cw], ones[:, :], buf[:, coff:coff + cw],
                        start=not started,
                        stop=(stop_set and i == len(chunks) - 1),
                    )
                    started = True
                inst.then_inc(pe_sem, 1)

            for k in range(1, NK):
                mmset(prs[k % 3], [(0, 2 * W3)], 7 + k)
            mmset(prs[0], [(0, S6), (W3, W3 + S6)], 13)
            mmset(prs[0], [(S6, W3), (W3 + S6, 2 * W3)], 14, stop_set=True)

    return nc


_CACHE = {}


def _get_nc():
    if "nc" not in _CACHE:
        _CACHE["nc"] = build_nc()
    return _CACHE["nc"]


def _perm_pad(arr, pad):
    out = np.full((ROWS, TP), pad, dtype=np.float32)
    out[:, :T] = arr
    out = out.reshape(RP, P, TP).transpose(1, 0, 2).reshape(P, FWH)
    return np.ascontiguousarray(out).astype(mybir.dt.np(_BF16))


def _run(actioness, actioness_2, **spmd_kwargs):
    nc = _get_nc()
    a0 = np.ascontiguousarray(actioness, dtype=np.float32)
    a2 = np.ascontiguousarray(actioness_2, dtype=np.float32)
    in_maps = []
    for c in range(N_CORES):
        sl = slice(c * ROWS, (c + 1) * ROWS)
        in_maps.append({
            "a0": _perm_pad(a0[sl], PAD0),
            "a2": _perm_pad(a2[sl], 0.0),
        })
    res = run_bass_kernel_spmd(nc, in_maps, list(range(N_CORES)), **spmd_kwargs)

    total = 0.0
    for r in res.results:
        total += float(r["res"][0, 0])

    # norm + clamped-edge corrections on host in fp64 (~4% of the FLOPs)
    a0d = a0.astype(np.float64)
    a2d = a2.astype(np.float64)
    total += E_THETA * float(np.sum(np.sqrt(np.sum((a0d - a2d) ** 2, axis=1))))

    def f(i, j):
        return np.exp(-np.abs(a0d[:, i] - a0d[:, j]) / 2.0) * np.abs(
            a2d[:, i] - a2d[:, j])

    for i in range(1, 6):
        total += (6 - i) * float(np.sum(f(i, 0)))
    for m in range(1, 4):
        total += (4 - m) * float(np.sum(f(T - 1 - m, T - 1)))
    return np.float32(total), res


def kernel(actioness, actioness_2):
    # one retry: the axon-tunneled device occasionally throws a transient
    # INTERNAL error on the first touch after another process closes
    try:
        out, _ = _run(actioness, actioness_2)
    except Exception:
        out, _ = _run(actioness, actioness_2)
    return out
